# revision 14
# baseline (speedup 1.0000x reference)
"""Cosine-similarity retrieval kernel for Trainium2 (8 NeuronCores, SPMD).

Computes out[q, n] = cos(query[q], support[n]) for query [2048, 512] and
support [50000, 512], out [2048, 50000] float32 — matching
torch.nn.CosineSimilarity semantics (dots / max(|q|*|s|, 1e-8)).

Strategy:
  * Shard support on the N axis: 8 shards of 6250 rows. Each core reads only
    its shard plus the (replicated, small) query set and writes its own
    [2048, 6250] column block of the output; the full output is assembled on
    the host — no device collective needed.
  * Rows are pre-normalized on the host (norms in float64), so the device
    kernel is a pure matmul Qn @ Sn^T; the PSUM result IS the cosine.
  * Both operands are staged transposed ([D, *]) so the contraction dim D
    lands on SBUF partitions; the matmul streams the support shard with the
    query tile as the stationary operand.
  * Matmul dtype is float32r (fp32 storage, ~12-bit mantissa in the PE) at
    full 1 cycle/row streaming rate; storage/DMA stays plain fp32.
"""

import os

import numpy as np

QN, DN, NN = 2048, 512, 50000
N_CORES = 8
NSH = NN // N_CORES  # 6250 support rows per core
P = 128
KT = DN // P  # 4 contraction chunks
QT = QN // P  # 16 query tiles
N_CHUNKS = 13  # per-core n tiling; 6250/13 keeps every matmul N >= 256
# n-chunk groups that share one PE weight load per (q-tile, k); each group's
# chunks accumulate in separate PSUM banks so the k-loop only reloads
# weights once per group.
GROUPS = [(0, 7), (7, 6)]
# qT is loaded in column chunks interleaved with the sT group loads so the
# first matmuls start as early as possible (q-tile 0 first, rest later).
QCHUNKS = [(0, 128), (128, 896), (1024, 1024)]
EPS = 1e-8

# "fp32r" (default): fp32 storage, float32r matmul (fast, ~2**-13 precision)
# "fp16": float16 storage+matmul. "fp32": exact fp32 matmul (4x slower PE).
DT_MODE = os.environ.get("COS_DT_MODE", "fp32r")

_PROGRAM = {}


def _chunks(total, n, granularity=1):
    # fp32r matmul requires an even moving free dim (ISA s3d3_mm_fp32r
    # restriction), so chunk at `granularity` then scale back up.
    assert total % granularity == 0
    units = total // granularity
    base, rem = divmod(units, n)
    sizes = [(base + 1) * granularity] * rem + [base * granularity] * (n - rem)
    out, start = [], 0
    for s in sizes:
        out.append((start, s))
        start += s
    return out


def _round_fp32r(x):
    """Round fp32 to the PE's float32r format: round-to-nearest-even keeping
    11 explicit mantissa bits (low 12 bits zeroed). Matches
    neuron_dtypes.fp32r.cast_fp32_to_fp32r for normal/zero values."""
    u = np.ascontiguousarray(x, dtype=np.float32).view(np.uint32)
    lsb = (u >> 12) & 1
    r = (u + np.uint32(0x7FF) + lsb) & np.uint32(0xFFFFF000)
    return r.view(np.float32)


def _patch_ldw_opt():
    """walrus's LDWEIGHTS dedup (--enable-ldw-opt) is hardcoded off in
    concourse; consecutive matmuls here share weights, so turn it on."""
    from concourse import bass_utils as bu

    if getattr(bu.run_command, "_ldw_patched", False):
        return
    orig = bu.run_command

    def patched(argv, **kwargs):
        if isinstance(argv, list) and "--enable-ldw-opt=false" in argv:
            argv = [
                "--enable-ldw-opt=true" if a == "--enable-ldw-opt=false" else a
                for a in argv
            ]
        return orig(argv, **kwargs)

    patched._ldw_patched = True
    bu.run_command = patched


def _build_program(dt_mode):
    import concourse.bass as bass  # noqa: F401
    import concourse.tile as tile
    from concourse import bacc, mybir

    if os.environ.get("COS_LDW_OPT", "1") != "0":
        _patch_ldw_opt()

    if dt_mode == "fp16":
        store_dt = mybir.dt.float16
    elif dt_mode == "fp32":
        store_dt = mybir.dt.float32
    else:
        # float32r end-to-end: DMA moves bits, host pre-rounds, and the
        # walrus verifier sees properly-rounded fp32r feeding the matmul.
        store_dt = mybir.dt.float32r

    nc = bacc.Bacc(
        "TRN2", target_bir_lowering=False, debug=False, num_devices=N_CORES
    )
    qT = nc.dram_tensor("qT", [DN, QN], store_dt, kind="ExternalInput").ap()
    sT = nc.dram_tensor("sT", [DN, NSH], store_dt, kind="ExternalInput").ap()
    out = nc.dram_tensor("out", [QN, NSH], mybir.dt.float32, kind="ExternalOutput").ap()

    chunks = _chunks(NSH, N_CHUNKS, granularity=2)

    with tile.TileContext(nc) as tc:
        with (
            tc.tile_pool(name="qw", bufs=1) as qpool,
            tc.tile_pool(name="sw", bufs=1) as spool,
            tc.tile_pool(name="ps", bufs=8, space="PSUM") as pspool,
            tc.tile_pool(name="ostage", bufs=4) as opool,
        ):
            # Load order follows first-pass consumption: q-tile-0 weights,
            # then sT for group 0, then more qT columns, then sT group 1...
            qts = {}  # (k, qchunk_idx) -> tile
            sts = {}  # (k, j) -> tile

            def load_qchunk(ci):
                c0, cw = QCHUNKS[ci]
                for k in range(KT):
                    t = qpool.tile(
                        [P, cw], store_dt, name=f"qTs{k}_{ci}", tag=f"qTs{k}_{ci}"
                    )
                    nc.sync.dma_start(t[:], qT[k * P : (k + 1) * P, c0 : c0 + cw])
                    qts[k, ci] = t

            def load_sgroup(g0, gn):
                for k in range(KT):
                    for j in range(g0, g0 + gn):
                        n0, nw = chunks[j]
                        t = spool.tile(
                            [P, nw], store_dt, name=f"sTs{k}_{j}", tag=f"sTs{k}_{j}"
                        )
                        nc.sync.dma_start(
                            t[:], sT[k * P : (k + 1) * P, n0 : n0 + nw]
                        )
                        sts[k, j] = t

            load_qchunk(0)
            for g, (g0, gn) in enumerate(GROUPS):
                load_sgroup(g0, gn)
                if g + 1 < len(QCHUNKS):
                    load_qchunk(g + 1)
            for ci in range(len(GROUPS), len(QCHUNKS)):
                load_qchunk(ci)

            def q_weight(k, qi):
                q0 = qi * P
                for ci, (c0, cw) in enumerate(QCHUNKS):
                    if c0 <= q0 < c0 + cw:
                        return qts[k, ci][:, q0 - c0 : q0 - c0 + P]
                raise AssertionError(qi)

            max_hw = max(
                sum(nw for _, nw in chunks[g0 + h0 : g0 + h1])
                for g0, gn in GROUPS
                for h0, h1 in ((0, gn // 2), (gn // 2, gn))
            )
            copy_idx = 0
            for qi in range(QT):
                for g0, gn in GROUPS:
                    group = chunks[g0 : g0 + gn]
                    pss = [
                        pspool.tile([P, 512], mybir.dt.float32, name="ps", tag="ps")
                        for _ in group
                    ]
                    # k outer, group inner: consecutive matmuls share lhsT so
                    # walrus's ldw-opt collapses their weight loads.
                    for k in range(KT):
                        w = q_weight(k, qi)
                        for gi, (n0, nw) in enumerate(group):
                            nc.tensor.matmul(
                                pss[gi][:, :nw],
                                lhsT=w,
                                rhs=sts[k, g0 + gi][:],
                                start=(k == 0),
                                stop=(k == KT - 1),
                            )
                    # two stores per group so the first launches after half
                    # the copies; copies alternate ACT/DVE.
                    for h0, h1 in ((0, gn // 2), (gn // 2, gn)):
                        ot = opool.tile(
                            [P, max_hw], mybir.dt.float32, name="ot", tag="ot"
                        )
                        off = 0
                        for gi in range(h0, h1):
                            nw = group[gi][1]
                            if copy_idx % 2 == 0:
                                nc.scalar.copy(
                                    out=ot[:, off : off + nw], in_=pss[gi][:, :nw]
                                )
                            else:
                                nc.vector.tensor_copy(
                                    out=ot[:, off : off + nw], in_=pss[gi][:, :nw]
                                )
                            copy_idx += 1
                            off += nw
                        nc.sync.dma_start(
                            out[
                                qi * P : (qi + 1) * P,
                                group[h0][0] : group[h0][0] + off,
                            ],
                            ot[:, :off],
                        )
    nc.compile()
    return nc


def _get_program(dt_mode=None):
    dt_mode = dt_mode or DT_MODE
    if dt_mode not in _PROGRAM:
        _PROGRAM[dt_mode] = _build_program(dt_mode)
    return _PROGRAM[dt_mode]


def _prep_inputs(support_set, query_set, dt_mode=None):
    dt_mode = dt_mode or DT_MODE
    S = np.asarray(support_set, dtype=np.float32)
    Q = np.asarray(query_set, dtype=np.float32)
    assert S.shape == (NN, DN) and Q.shape == (QN, DN)

    host_dt = np.float16 if dt_mode == "fp16" else np.float32

    def normalize(x):
        x64 = x.astype(np.float64)
        norm = np.sqrt(np.einsum("nd,nd->n", x64, x64))
        # Reference divides by max(|q|*|s|, eps). Norms here are ~22, so the
        # eps clamp never binds for real rows; an all-zero row would give
        # dots == 0 in the reference too, so map inv-norm to 0 there.
        inv = np.where(norm > 0, 1.0 / np.maximum(norm, EPS), 0.0)
        return (x64 * inv[:, None]).astype(host_dt)

    Sn = normalize(S)
    Qn = normalize(Q)
    if dt_mode == "fp32r":
        Sn = _round_fp32r(Sn)
        Qn = _round_fp32r(Qn)
    qT = np.ascontiguousarray(Qn.T)  # [512, 2048]
    in_maps = []
    for c in range(N_CORES):
        sT = np.ascontiguousarray(Sn[c * NSH : (c + 1) * NSH].T)  # [512, 6250]
        in_maps.append({"qT": qT, "sT": sT})
    return in_maps


def _run(in_maps, dt_mode=None, trace=False, **kwargs):
    from concourse import bass_utils

    nc = _get_program(dt_mode)
    return bass_utils.run_bass_kernel_spmd(
        nc, in_maps, core_ids=list(range(N_CORES)), trace=trace, **kwargs
    )


def _assemble(results):
    return np.concatenate(
        [results[c]["out"] for c in range(N_CORES)], axis=1
    )


def kernel(support_set, query_set):
    in_maps = _prep_inputs(support_set, query_set)
    res = _run(in_maps)
    return _assemble(res.results)


# revision 18
# speedup vs baseline: 1.0722x; 1.0722x over previous
"""Cosine-similarity retrieval kernel for Trainium2 (8 NeuronCores, SPMD).

Computes out[q, n] = cos(query[q], support[n]) for query [2048, 512] and
support [50000, 512], out [2048, 50000] float32 — matching
torch.nn.CosineSimilarity semantics (dots / max(|q|*|s|, 1e-8)).

Strategy:
  * Shard support on the N axis: 8 shards of 6250 rows. Each core reads only
    its shard plus the (replicated, small) query set and writes its own
    [2048, 6250] column block of the output; the full output is assembled on
    the host — no device collective needed.
  * Rows are pre-normalized on the host (norms in float64), so the device
    kernel is a pure matmul Qn @ Sn^T; the PSUM result IS the cosine.
  * Both operands are staged transposed ([D, *]) so the contraction dim D
    lands on SBUF partitions; the matmul streams the support shard with the
    query tile as the stationary operand.
  * Matmul dtype is float32r (fp32 storage, ~12-bit mantissa in the PE) at
    full 1 cycle/row streaming rate; storage/DMA stays plain fp32.
"""

import os

import numpy as np

QN, DN, NN = 2048, 512, 50000
N_CORES = 8
NSH = NN // N_CORES  # 6250 support rows per core
P = 128
KT = DN // P  # 4 contraction chunks
QT = QN // P  # 16 query tiles
N_CHUNKS = 13  # per-core n tiling; 6250/13 keeps every matmul N >= 256
# qT is loaded in column chunks interleaved with the sT loads so the first
# matmuls start as early as possible (q-tile 0 first, rest later).
QCHUNKS = [(0, 128), (128, 896), (1024, 1024)]
EPS = 1e-8

# "fp32r" (default): fp32 storage, float32r matmul (fast, ~2**-13 precision)
# "fp16": float16 storage+matmul. "fp32": exact fp32 matmul (4x slower PE).
DT_MODE = os.environ.get("COS_DT_MODE", "fp32r")

_PROGRAM = {}


def _chunks(total, n, granularity=1):
    # fp32r matmul requires an even moving free dim (ISA s3d3_mm_fp32r
    # restriction), so chunk at `granularity` then scale back up.
    assert total % granularity == 0
    units = total // granularity
    base, rem = divmod(units, n)
    sizes = [(base + 1) * granularity] * rem + [base * granularity] * (n - rem)
    out, start = [], 0
    for s in sizes:
        out.append((start, s))
        start += s
    return out


def _round_fp32r(x):
    """Round fp32 to the PE's float32r format: round-to-nearest-even keeping
    11 explicit mantissa bits (low 12 bits zeroed). Matches
    neuron_dtypes.fp32r.cast_fp32_to_fp32r for normal/zero values."""
    u = np.ascontiguousarray(x, dtype=np.float32).view(np.uint32)
    lsb = (u >> 12) & 1
    r = (u + np.uint32(0x7FF) + lsb) & np.uint32(0xFFFFF000)
    return r.view(np.float32)


def _patch_ldw_opt():
    """walrus's LDWEIGHTS dedup (--enable-ldw-opt) is hardcoded off in
    concourse; consecutive matmuls here share weights, so turn it on."""
    from concourse import bass_utils as bu

    if getattr(bu.run_command, "_ldw_patched", False):
        return
    orig = bu.run_command

    def patched(argv, **kwargs):
        if isinstance(argv, list) and "--enable-ldw-opt=false" in argv:
            argv = [
                "--enable-ldw-opt=true" if a == "--enable-ldw-opt=false" else a
                for a in argv
            ]
        return orig(argv, **kwargs)

    patched._ldw_patched = True
    bu.run_command = patched


def _build_program(dt_mode):
    import concourse.bass as bass  # noqa: F401
    import concourse.tile as tile
    from concourse import bacc, mybir

    if os.environ.get("COS_LDW_OPT", "1") != "0":
        _patch_ldw_opt()

    if dt_mode == "fp16":
        store_dt = mybir.dt.float16
    elif dt_mode == "fp32":
        store_dt = mybir.dt.float32
    else:
        # float32r end-to-end: DMA moves bits, host pre-rounds, and the
        # walrus verifier sees properly-rounded fp32r feeding the matmul.
        store_dt = mybir.dt.float32r

    nc = bacc.Bacc(
        "TRN2", target_bir_lowering=False, debug=False, num_devices=N_CORES
    )
    qT = nc.dram_tensor("qT", [DN, QN], store_dt, kind="ExternalInput").ap()
    sT = nc.dram_tensor("sT", [DN, NSH], store_dt, kind="ExternalInput").ap()
    out = nc.dram_tensor("out", [QN, NSH], mybir.dt.float32, kind="ExternalOutput").ap()

    chunks = _chunks(NSH, N_CHUNKS, granularity=2)

    with tile.TileContext(nc) as tc:
        with (
            tc.tile_pool(name="qw", bufs=1) as qpool,
            tc.tile_pool(name="sw", bufs=1) as spool,
            tc.tile_pool(name="ps", bufs=8, space="PSUM") as pspool,
            tc.tile_pool(name="ostage", bufs=6) as opool,
        ):
            # Load order follows first-pass consumption: q-tile-0 weights,
            # then sT for group 0, then more qT columns, then sT group 1...
            qts = {}  # (k, qchunk_idx) -> tile
            sts = {}  # (k, j) -> tile

            def load_qchunk(ci):
                c0, cw = QCHUNKS[ci]
                for k in range(KT):
                    t = qpool.tile(
                        [P, cw], store_dt, name=f"qTs{k}_{ci}", tag=f"qTs{k}_{ci}"
                    )
                    nc.sync.dma_start(t[:], qT[k * P : (k + 1) * P, c0 : c0 + cw])
                    qts[k, ci] = t

            def load_schunk(j):
                n0, nw = chunks[j]
                for k in range(KT):
                    t = spool.tile(
                        [P, nw], store_dt, name=f"sTs{k}_{j}", tag=f"sTs{k}_{j}"
                    )
                    nc.sync.dma_start(t[:], sT[k * P : (k + 1) * P, n0 : n0 + nw])
                    sts[k, j] = t

            load_qchunk(0)
            for j in range(N_CHUNKS):
                load_schunk(j)
                if j == 1:
                    load_qchunk(1)
                elif j == 5:
                    load_qchunk(2)

            def q_weight(k, qi):
                q0 = qi * P
                for ci, (c0, cw) in enumerate(QCHUNKS):
                    if c0 <= q0 < c0 + cw:
                        return qts[k, ci][:, q0 - c0 : q0 - c0 + P]
                raise AssertionError(qi)

            copy_idx = 0
            for qi in range(QT):
                for j, (n0, nw) in enumerate(chunks):
                    ps = pspool.tile([P, 512], mybir.dt.float32, name="ps", tag="ps")
                    for k in range(KT):
                        nc.tensor.matmul(
                            ps[:, :nw],
                            lhsT=q_weight(k, qi),
                            rhs=sts[k, j][:],
                            start=(k == 0),
                            stop=(k == KT - 1),
                        )
                    ot = opool.tile([P, 512], mybir.dt.float32, name="ot", tag="ot")
                    # split PSUM->SBUF copies between ACT and DVE
                    if copy_idx % 2 == 0:
                        nc.scalar.copy(out=ot[:, :nw], in_=ps[:, :nw])
                    else:
                        nc.vector.tensor_copy(out=ot[:, :nw], in_=ps[:, :nw])
                    copy_idx += 1
                    nc.sync.dma_start(
                        out[qi * P : (qi + 1) * P, n0 : n0 + nw], ot[:, :nw]
                    )
    nc.compile()
    return nc


def _get_program(dt_mode=None):
    dt_mode = dt_mode or DT_MODE
    if dt_mode not in _PROGRAM:
        _PROGRAM[dt_mode] = _build_program(dt_mode)
    return _PROGRAM[dt_mode]


def _prep_inputs(support_set, query_set, dt_mode=None):
    dt_mode = dt_mode or DT_MODE
    S = np.asarray(support_set, dtype=np.float32)
    Q = np.asarray(query_set, dtype=np.float32)
    assert S.shape == (NN, DN) and Q.shape == (QN, DN)

    host_dt = np.float16 if dt_mode == "fp16" else np.float32

    def normalize(x):
        x64 = x.astype(np.float64)
        norm = np.sqrt(np.einsum("nd,nd->n", x64, x64))
        # Reference divides by max(|q|*|s|, eps). Norms here are ~22, so the
        # eps clamp never binds for real rows; an all-zero row would give
        # dots == 0 in the reference too, so map inv-norm to 0 there.
        inv = np.where(norm > 0, 1.0 / np.maximum(norm, EPS), 0.0)
        return (x64 * inv[:, None]).astype(host_dt)

    Sn = normalize(S)
    Qn = normalize(Q)
    if dt_mode == "fp32r":
        Sn = _round_fp32r(Sn)
        Qn = _round_fp32r(Qn)
    qT = np.ascontiguousarray(Qn.T)  # [512, 2048]
    in_maps = []
    for c in range(N_CORES):
        sT = np.ascontiguousarray(Sn[c * NSH : (c + 1) * NSH].T)  # [512, 6250]
        in_maps.append({"qT": qT, "sT": sT})
    return in_maps


def _run(in_maps, dt_mode=None, trace=False, **kwargs):
    from concourse import bass_utils

    nc = _get_program(dt_mode)
    return bass_utils.run_bass_kernel_spmd(
        nc, in_maps, core_ids=list(range(N_CORES)), trace=trace, **kwargs
    )


def _assemble(results):
    return np.concatenate(
        [results[c]["out"] for c in range(N_CORES)], axis=1
    )


def kernel(support_set, query_set):
    in_maps = _prep_inputs(support_set, query_set)
    res = _run(in_maps)
    return _assemble(res.results)


# revision 22
# speedup vs baseline: 1.1630x; 1.0847x over previous
"""Cosine-similarity retrieval kernel for Trainium2 (8 NeuronCores, SPMD).

Computes out[q, n] = cos(query[q], support[n]) for query [2048, 512] and
support [50000, 512], out [2048, 50000] float32 — matching
torch.nn.CosineSimilarity semantics (dots / max(|q|*|s|, 1e-8)).

Strategy:
  * Shard support on the N axis: 8 shards of 6250 rows. Each core reads only
    its shard plus the (replicated, small) query set and writes its own
    [2048, 6250] column block of the output; the full output is assembled on
    the host — no device collective needed.
  * Rows are pre-normalized on the host (norms in float64), so the device
    kernel is a pure matmul Qn @ Sn^T; the PSUM result IS the cosine.
  * Both operands are staged transposed ([D, *]) so the contraction dim D
    lands on SBUF partitions; the matmul streams the support shard with the
    query tile as the stationary operand.
  * Matmul dtype is float32r (fp32 storage, ~12-bit mantissa in the PE) at
    full 1 cycle/row streaming rate; storage/DMA stays plain fp32.
"""

import os

import numpy as np

QN, DN, NN = 2048, 512, 50000
N_CORES = 8
NSH = NN // N_CORES  # 6250 support rows per core
P = 128
KT = DN // P  # 4 contraction chunks
QT = QN // P  # 16 query tiles
N_CHUNKS = 13  # per-core n tiling; 6250/13 keeps every matmul N >= 256
# qT is loaded in column chunks interleaved with the first sT loads so the
# first matmuls start as early as possible.
QCHUNKS = [(c, 512) for c in range(0, QN, 512)]
ST_PREFETCH = 3  # sT chunk double-buffer depth in the j-outer loop
EPS = 1e-8

# "fp32r" (default): fp32 storage, float32r matmul (fast, ~2**-13 precision)
# "fp16": float16 storage+matmul. "fp32": exact fp32 matmul (4x slower PE).
DT_MODE = os.environ.get("COS_DT_MODE", "fp32r")
# Output staged as fp16 (halves the dominant HBM write traffic; host upcasts
# to f32; adds ~2.8e-4 L2 quantization). "fp32" restores exact staging.
OUT_MODE = os.environ.get("COS_OUT_DT", "fp16")

_PROGRAM = {}


def _chunks(total, n, granularity=1):
    # fp32r matmul requires an even moving free dim (ISA s3d3_mm_fp32r
    # restriction), so chunk at `granularity` then scale back up.
    assert total % granularity == 0
    units = total // granularity
    base, rem = divmod(units, n)
    sizes = [(base + 1) * granularity] * rem + [base * granularity] * (n - rem)
    out, start = [], 0
    for s in sizes:
        out.append((start, s))
        start += s
    return out


def _round_fp32r(x):
    """Round fp32 to the PE's float32r format: round-to-nearest-even keeping
    11 explicit mantissa bits (low 12 bits zeroed). Matches
    neuron_dtypes.fp32r.cast_fp32_to_fp32r for normal/zero values."""
    u = np.ascontiguousarray(x, dtype=np.float32).view(np.uint32)
    lsb = (u >> 12) & 1
    r = (u + np.uint32(0x7FF) + lsb) & np.uint32(0xFFFFF000)
    return r.view(np.float32)


def _patch_ldw_opt():
    """walrus's LDWEIGHTS dedup (--enable-ldw-opt) is hardcoded off in
    concourse; consecutive matmuls here share weights, so turn it on."""
    from concourse import bass_utils as bu

    if getattr(bu.run_command, "_ldw_patched", False):
        return
    orig = bu.run_command

    def patched(argv, **kwargs):
        if isinstance(argv, list) and "--enable-ldw-opt=false" in argv:
            argv = [
                "--enable-ldw-opt=true" if a == "--enable-ldw-opt=false" else a
                for a in argv
            ]
        return orig(argv, **kwargs)

    patched._ldw_patched = True
    bu.run_command = patched


def _build_program(dt_mode, out_mode):
    import concourse.bass as bass  # noqa: F401
    import concourse.tile as tile
    from concourse import bacc, mybir

    if os.environ.get("COS_LDW_OPT", "1") != "0":
        _patch_ldw_opt()

    if dt_mode == "fp16":
        store_dt = mybir.dt.float16
    elif dt_mode == "fp32":
        store_dt = mybir.dt.float32
    else:
        # float32r end-to-end: DMA moves bits, host pre-rounds, and the
        # walrus verifier sees properly-rounded fp32r feeding the matmul.
        store_dt = mybir.dt.float32r
    out_dt = mybir.dt.float16 if out_mode == "fp16" else mybir.dt.float32

    nc = bacc.Bacc(
        "TRN2", target_bir_lowering=False, debug=False, num_devices=N_CORES
    )
    qT = nc.dram_tensor("qT", [DN, QN], store_dt, kind="ExternalInput").ap()
    sT = nc.dram_tensor("sT", [DN, NSH], store_dt, kind="ExternalInput").ap()
    out = nc.dram_tensor("out", [QN, NSH], out_dt, kind="ExternalOutput").ap()

    chunks = _chunks(NSH, N_CHUNKS, granularity=2)

    with tile.TileContext(nc) as tc:
        with (
            tc.tile_pool(name="qw", bufs=1) as qpool,
            tc.tile_pool(name="sw", bufs=1) as spool,
            tc.tile_pool(name="ps", bufs=8, space="PSUM") as pspool,
            tc.tile_pool(name="ostage", bufs=8) as opool,
        ):
            qts = {}  # (k, qchunk_idx) -> resident tile
            sts = {}  # (k, j) -> cycling tile

            def load_qchunk(ci):
                c0, cw = QCHUNKS[ci]
                for k in range(KT):
                    t = qpool.tile(
                        [P, cw], store_dt, name=f"qTs{k}_{ci}", tag=f"qTs{k}_{ci}"
                    )
                    nc.sync.dma_start(t[:], qT[k * P : (k + 1) * P, c0 : c0 + cw])
                    qts[k, ci] = t

            def load_schunk(j):
                # per-k tags share ST_PREFETCH+2 slots: the j loop streams
                # sT chunks through SBUF instead of keeping all resident
                n0, nw = chunks[j]
                for k in range(KT):
                    t = spool.tile(
                        [P, 482],
                        store_dt,
                        name=f"sTs{k}_{j}",
                        tag=f"sTs{k}",
                        bufs=ST_PREFETCH + 2,
                    )
                    nc.sync.dma_start(
                        t[:, :nw], sT[k * P : (k + 1) * P, n0 : n0 + nw]
                    )
                    sts[k, j] = t

            # interleave qT column-chunk loads with the first sT chunks so
            # the j=0 pass (which sweeps all q-tiles) isn't weight-starved
            load_qchunk(0)
            for j in range(ST_PREFETCH):
                load_schunk(j)
                if j + 1 < len(QCHUNKS):
                    load_qchunk(j + 1)
            for ci in range(ST_PREFETCH, len(QCHUNKS)):
                load_qchunk(ci)

            def q_weight(k, qi):
                q0 = qi * P
                ci, off = divmod(q0, 512)
                return qts[k, ci][:, off : off + P]

            copy_idx = 0
            # j outer / q inner: each j-pass reuses one ~1MB sT chunk for
            # all 16 q-tiles, so the DMA feed never starves the PE (the
            # q-outer order would need the whole shard per pass).
            for j, (n0, nw) in enumerate(chunks):
                if j + ST_PREFETCH < N_CHUNKS:
                    load_schunk(j + ST_PREFETCH)
                for qi in range(QT):
                    ps = pspool.tile([P, 512], mybir.dt.float32, name="ps", tag="ps")
                    for k in range(KT):
                        nc.tensor.matmul(
                            ps[:, :nw],
                            lhsT=q_weight(k, qi),
                            rhs=sts[k, j][:, :nw],
                            start=(k == 0),
                            stop=(k == KT - 1),
                        )
                    ot = opool.tile([P, 512], out_dt, name="ot", tag="ot")
                    # split PSUM->SBUF copies (with fp16 downcast) ACT/DVE
                    if copy_idx % 2 == 0:
                        nc.scalar.copy(out=ot[:, :nw], in_=ps[:, :nw])
                    else:
                        nc.vector.tensor_copy(out=ot[:, :nw], in_=ps[:, :nw])
                    copy_idx += 1
                    nc.sync.dma_start(
                        out[qi * P : (qi + 1) * P, n0 : n0 + nw], ot[:, :nw]
                    )
    nc.compile()
    return nc


def _get_program(dt_mode=None, out_mode=None):
    key = (dt_mode or DT_MODE, out_mode or OUT_MODE)
    if key not in _PROGRAM:
        _PROGRAM[key] = _build_program(*key)
    return _PROGRAM[key]


def _prep_inputs(support_set, query_set, dt_mode=None):
    dt_mode = dt_mode or DT_MODE
    S = np.asarray(support_set, dtype=np.float32)
    Q = np.asarray(query_set, dtype=np.float32)
    assert S.shape == (NN, DN) and Q.shape == (QN, DN)

    host_dt = np.float16 if dt_mode == "fp16" else np.float32

    def normalize(x):
        x64 = x.astype(np.float64)
        norm = np.sqrt(np.einsum("nd,nd->n", x64, x64))
        # Reference divides by max(|q|*|s|, eps). Norms here are ~22, so the
        # eps clamp never binds for real rows; an all-zero row would give
        # dots == 0 in the reference too, so map inv-norm to 0 there.
        inv = np.where(norm > 0, 1.0 / np.maximum(norm, EPS), 0.0)
        return (x64 * inv[:, None]).astype(host_dt)

    Sn = normalize(S)
    Qn = normalize(Q)
    if dt_mode == "fp32r":
        Sn = _round_fp32r(Sn)
        Qn = _round_fp32r(Qn)
    qT = np.ascontiguousarray(Qn.T)  # [512, 2048]
    in_maps = []
    for c in range(N_CORES):
        sT = np.ascontiguousarray(Sn[c * NSH : (c + 1) * NSH].T)  # [512, 6250]
        in_maps.append({"qT": qT, "sT": sT})
    return in_maps


def _run(in_maps, dt_mode=None, out_mode=None, trace=False, **kwargs):
    from concourse import bass_utils

    nc = _get_program(dt_mode, out_mode)
    return bass_utils.run_bass_kernel_spmd(
        nc, in_maps, core_ids=list(range(N_CORES)), trace=trace, **kwargs
    )


def _assemble(results):
    return np.concatenate(
        [np.asarray(results[c]["out"], dtype=np.float32) for c in range(N_CORES)],
        axis=1,
    )


def kernel(support_set, query_set):
    in_maps = _prep_inputs(support_set, query_set)
    res = _run(in_maps)
    return _assemble(res.results)


# revision 24
# speedup vs baseline: 1.2314x; 1.0588x over previous
"""Cosine-similarity retrieval kernel for Trainium2 (8 NeuronCores, SPMD).

Computes out[q, n] = cos(query[q], support[n]) for query [2048, 512] and
support [50000, 512], out [2048, 50000] float32 — matching
torch.nn.CosineSimilarity semantics (dots / max(|q|*|s|, 1e-8)).

Strategy:
  * Shard support on the N axis: 8 shards of 6250 rows. Each core reads only
    its shard plus the (replicated, small) query set and writes its own
    [2048, 6250] column block of the output; the full output is assembled on
    the host — no device collective needed.
  * Rows are pre-normalized on the host (norms in float64), so the device
    kernel is a pure matmul Qn @ Sn^T; the PSUM result IS the cosine.
  * Both operands are staged transposed ([D, *]) so the contraction dim D
    lands on SBUF partitions; the matmul streams the support shard with the
    query tile as the stationary operand.
  * Matmul dtype is float32r (fp32 storage, ~12-bit mantissa in the PE) at
    full 1 cycle/row streaming rate; storage/DMA stays plain fp32.
"""

import os

import numpy as np

QN, DN, NN = 2048, 512, 50000
N_CORES = 8
NSH = NN // N_CORES  # 6250 support rows per core
P = 128
KT = DN // P  # 4 contraction chunks
QT = QN // P  # 16 query tiles
N_CHUNKS = 13  # per-core n tiling; 6250/13 keeps every matmul N >= 256
# qT is loaded in column chunks interleaved with the first sT loads so the
# first matmuls start as early as possible.
QCHUNKS = [(c, 512) for c in range(0, QN, 512)]
ST_PREFETCH = 4  # sT chunk double-buffer depth in the j-outer loop
QBATCH = 4  # q-tiles per merged output store
EPS = 1e-8

# "fp32r" (default): fp32 storage, float32r matmul (fast, ~2**-13 precision)
# "fp16": float16 storage+matmul. "fp32": exact fp32 matmul (4x slower PE).
DT_MODE = os.environ.get("COS_DT_MODE", "fp32r")
# Output staged as fp16 (halves the dominant HBM write traffic; host upcasts
# to f32; adds ~2.8e-4 L2 quantization). "fp32" restores exact staging.
OUT_MODE = os.environ.get("COS_OUT_DT", "fp16")

_PROGRAM = {}


def _chunks(total, n, granularity=1):
    # fp32r matmul requires an even moving free dim (ISA s3d3_mm_fp32r
    # restriction), so chunk at `granularity` then scale back up.
    assert total % granularity == 0
    units = total // granularity
    base, rem = divmod(units, n)
    sizes = [(base + 1) * granularity] * rem + [base * granularity] * (n - rem)
    out, start = [], 0
    for s in sizes:
        out.append((start, s))
        start += s
    return out


def _round_fp32r(x):
    """Round fp32 to the PE's float32r format: round-to-nearest-even keeping
    11 explicit mantissa bits (low 12 bits zeroed). Matches
    neuron_dtypes.fp32r.cast_fp32_to_fp32r for normal/zero values."""
    u = np.ascontiguousarray(x, dtype=np.float32).view(np.uint32)
    lsb = (u >> 12) & 1
    r = (u + np.uint32(0x7FF) + lsb) & np.uint32(0xFFFFF000)
    return r.view(np.float32)


def _patch_ldw_opt():
    """walrus's LDWEIGHTS dedup (--enable-ldw-opt) is hardcoded off in
    concourse; consecutive matmuls here share weights, so turn it on."""
    from concourse import bass_utils as bu

    if getattr(bu.run_command, "_ldw_patched", False):
        return
    orig = bu.run_command

    def patched(argv, **kwargs):
        if isinstance(argv, list) and "--enable-ldw-opt=false" in argv:
            argv = [
                "--enable-ldw-opt=true" if a == "--enable-ldw-opt=false" else a
                for a in argv
            ]
        return orig(argv, **kwargs)

    patched._ldw_patched = True
    bu.run_command = patched


def _build_program(dt_mode, out_mode):
    import concourse.bass as bass  # noqa: F401
    import concourse.tile as tile
    from concourse import bacc, mybir

    if os.environ.get("COS_LDW_OPT", "1") != "0":
        _patch_ldw_opt()

    if dt_mode == "fp16":
        store_dt = mybir.dt.float16
    elif dt_mode == "fp32":
        store_dt = mybir.dt.float32
    else:
        # float32r end-to-end: DMA moves bits, host pre-rounds, and the
        # walrus verifier sees properly-rounded fp32r feeding the matmul.
        store_dt = mybir.dt.float32r
    out_dt = mybir.dt.float16 if out_mode == "fp16" else mybir.dt.float32

    nc = bacc.Bacc(
        "TRN2", target_bir_lowering=False, debug=False, num_devices=N_CORES
    )
    qT = nc.dram_tensor("qT", [DN, QN], store_dt, kind="ExternalInput").ap()
    sT = nc.dram_tensor("sT", [DN, NSH], store_dt, kind="ExternalInput").ap()
    out = nc.dram_tensor("out", [QN, NSH], out_dt, kind="ExternalOutput").ap()

    chunks = _chunks(NSH, N_CHUNKS, granularity=2)
    max_nw = max(nw for _, nw in chunks)

    # 3D views putting the contraction (k) / q-tile (g) index on a middle
    # axis so one DMA instruction moves all 4 k-slices of a chunk (or all
    # QBATCH q-tiles of a store) — each dma_start costs ~0.6us of Sync
    # issue time, so instruction count matters.
    qT3 = qT.rearrange("(k p) q -> p k q", p=P)  # [128, KT, QN]
    sT3 = sT.rearrange("(k p) n -> p k n", p=P)  # [128, KT, NSH]
    out3 = out.rearrange("(g p) n -> p g n", p=P)  # [128, QT, NSH]

    with tile.TileContext(nc) as tc:
        with (
            tc.tile_pool(name="qw", bufs=1) as qpool,
            tc.tile_pool(name="sw", bufs=1) as spool,
            tc.tile_pool(name="ps", bufs=8, space="PSUM") as pspool,
            tc.tile_pool(name="ostage", bufs=4) as opool,
        ):
            qts = {}  # qchunk_idx -> resident [P, KT, 512] tile
            sts = {}  # j -> cycling [P, KT, max_nw] tile

            def load_qchunk(ci, split=False):
                c0, cw = QCHUNKS[ci]
                t = qpool.tile([P, KT, cw], store_dt, name=f"qTs{ci}", tag=f"qTs{ci}")
                if split:  # per-k pieces so the first matmul starts sooner
                    for k in range(KT):
                        nc.sync.dma_start(t[:, k, :], qT3[:, k, c0 : c0 + cw])
                else:
                    t_ap = t[:]
                    nc.sync.dma_start(t_ap, qT3[:, :, c0 : c0 + cw])
                qts[ci] = t

            def load_schunk(j, split=False):
                n0, nw = chunks[j]
                t = spool.tile(
                    [P, KT, max_nw],
                    store_dt,
                    name=f"sTs{j}",
                    tag="sTs",
                    bufs=ST_PREFETCH + 2,
                )
                if split:
                    for k in range(KT):
                        nc.sync.dma_start(t[:, k, :nw], sT3[:, k, n0 : n0 + nw])
                else:
                    nc.sync.dma_start(t[:, :, :nw], sT3[:, :, n0 : n0 + nw])
                sts[j] = t

            # interleave qT column-chunk loads with the first sT chunks so
            # the j=0 pass (which sweeps all q-tiles) isn't weight-starved
            load_qchunk(0, split=True)
            load_schunk(0, split=True)
            qc_next = 1
            for j in range(1, ST_PREFETCH):
                load_schunk(j)
                if qc_next < len(QCHUNKS):
                    load_qchunk(qc_next)
                    qc_next += 1
            while qc_next < len(QCHUNKS):
                load_qchunk(qc_next)
                qc_next += 1

            def q_weight(k, qi):
                ci, off = divmod(qi * P, 512)
                return qts[ci][:, k, off : off + P]

            copy_idx = 0
            # j outer / q inner: each j-pass reuses one ~1MB sT chunk for
            # all 16 q-tiles, so the DMA feed never starves the PE (the
            # q-outer order would need the whole shard per pass).
            for j, (n0, nw) in enumerate(chunks):
                if j + ST_PREFETCH < N_CHUNKS:
                    load_schunk(j + ST_PREFETCH)
                for qg in range(QT // QBATCH):
                    ot = opool.tile([P, QBATCH, max_nw], out_dt, name="ot", tag="ot")
                    for qb in range(QBATCH):
                        qi = qg * QBATCH + qb
                        ps = pspool.tile(
                            [P, 512], mybir.dt.float32, name="ps", tag="ps"
                        )
                        for k in range(KT):
                            nc.tensor.matmul(
                                ps[:, :nw],
                                lhsT=q_weight(k, qi),
                                rhs=sts[j][:, k, :nw],
                                start=(k == 0),
                                stop=(k == KT - 1),
                            )
                        # split PSUM->SBUF copies (with downcast) ACT/DVE
                        if copy_idx % 2 == 0:
                            nc.scalar.copy(out=ot[:, qb, :nw], in_=ps[:, :nw])
                        else:
                            nc.vector.tensor_copy(out=ot[:, qb, :nw], in_=ps[:, :nw])
                        copy_idx += 1
                    nc.sync.dma_start(
                        out3[:, qg * QBATCH : (qg + 1) * QBATCH, n0 : n0 + nw],
                        ot[:, :, :nw],
                    )
    nc.compile()
    return nc


def _get_program(dt_mode=None, out_mode=None):
    key = (dt_mode or DT_MODE, out_mode or OUT_MODE)
    if key not in _PROGRAM:
        _PROGRAM[key] = _build_program(*key)
    return _PROGRAM[key]


def _prep_inputs(support_set, query_set, dt_mode=None):
    dt_mode = dt_mode or DT_MODE
    S = np.asarray(support_set, dtype=np.float32)
    Q = np.asarray(query_set, dtype=np.float32)
    assert S.shape == (NN, DN) and Q.shape == (QN, DN)

    host_dt = np.float16 if dt_mode == "fp16" else np.float32

    def normalize(x):
        x64 = x.astype(np.float64)
        norm = np.sqrt(np.einsum("nd,nd->n", x64, x64))
        # Reference divides by max(|q|*|s|, eps). Norms here are ~22, so the
        # eps clamp never binds for real rows; an all-zero row would give
        # dots == 0 in the reference too, so map inv-norm to 0 there.
        inv = np.where(norm > 0, 1.0 / np.maximum(norm, EPS), 0.0)
        return (x64 * inv[:, None]).astype(host_dt)

    Sn = normalize(S)
    Qn = normalize(Q)
    if dt_mode == "fp32r":
        Sn = _round_fp32r(Sn)
        Qn = _round_fp32r(Qn)
    qT = np.ascontiguousarray(Qn.T)  # [512, 2048]
    in_maps = []
    for c in range(N_CORES):
        sT = np.ascontiguousarray(Sn[c * NSH : (c + 1) * NSH].T)  # [512, 6250]
        in_maps.append({"qT": qT, "sT": sT})
    return in_maps


def _run(in_maps, dt_mode=None, out_mode=None, trace=False, **kwargs):
    from concourse import bass_utils

    nc = _get_program(dt_mode, out_mode)
    return bass_utils.run_bass_kernel_spmd(
        nc, in_maps, core_ids=list(range(N_CORES)), trace=trace, **kwargs
    )


def _assemble(results):
    return np.concatenate(
        [np.asarray(results[c]["out"], dtype=np.float32) for c in range(N_CORES)],
        axis=1,
    )


def kernel(support_set, query_set):
    in_maps = _prep_inputs(support_set, query_set)
    res = _run(in_maps)
    return _assemble(res.results)
